# revision 21
# baseline (speedup 1.0000x reference)
"""ContrastiveDist kernel for TRN2 (8 NeuronCores, SPMD) -- v4.4.

out[n] = sum_e -(t_e . v_n) / (||t_e|| * ||v_n|| + eps)
       = (s . v_n) / ||v_n||      with s = -sum_e t_e / ||t_e||
(eps shifts the result by ~4e-11 relative -- dropped.)

Schedule design (from the v3 / v4.0-v4.3 traces):
 * THREE DMA queues: SP HWDGE (nc.sync), ACT HWDGE (nc.scalar), GPSIMD
   SWDGE (nc.gpsimd); ~285 GB/s aggregate, round-robin per PACKET so
   wider chunks (bigger per-partition runs) get a bigger share.  tgt
   quarters head both HWDGE rings; the GP ring issues eye+block0 first,
   then holds its bulk chunks behind the phase-A gpsimd square so tgt
   keeps most of the early bandwidth.
 * target entity-major [128e, 16, 256d], 4 quarters.  tgt-ssq runs on
   THREE engines in parallel (the serial-DVE version gated s by ~7us):
   q0/q1 DVE square+reduce, q2 GPSIMD square -> DVE reduce, q3 ACT
   Square+accum_out per tile.  ACT Abs_reciprocal_sqrt emits winv in
   BF16 directly; phase-A matmuls are EMITTED in expected-readiness
   order [q0, q1, q3, q2] (PE executes its stream in order).
 * s sign folds into the ACT Copy(scale=-1) psum->sbuf column copies;
   s_bf copies split per d-half so each colmm starts early.
 * Abs_reciprocal_sqrt shares its ACT table with Square and Copy -> one
   ACT_TABLE_LOAD, no DVE RECIPROCAL anywhere.
 * psum pairs grouped by ARRIVAL: pair0 = blocks 0-6 (early), pair1 =
   7-13; block-diagonal eye lhsT routes block b to its psum row; tails
   are ACT arsqrt [7,448] + one DVE mul, outs on the SP ring.
 * tile_wait_until logical timestamps align the static per-engine
   streams with the expected timeline (the Tile scheduler's own cost
   model mispredicts DMA arrivals).
 * fused DVE tensor_tensor_reduce is NOT used anywhere: it locks up
   the hardware (v4.0 bisect).
"""

import numpy as np
import ml_dtypes
from contextlib import ExitStack

import concourse.bacc as bacc
import concourse.bass as bass
import concourse.mybir as mybir
import concourse.tile as tile
from concourse import bass_utils

E, D = 2048, 256          # entities, embed dim
N_FULL = 50000            # total nodes
N_CORES = 8
NPC = N_FULL // N_CORES   # 6250 true nodes per core
G = 448                   # node columns per psum block (fp32 bank width)
NG = 14                   # node column blocks -> NPAD = 6272
NPAD = G * NG
NP = 7                    # blocks per psum pair
A = 2                     # d-halves (256 = 2*128 partitions)
ET = E // 128             # 16 entity tiles [128, 256]
EYC = NP * NP             # eye columns at the head of vt (49)
NC2 = EYC + NPAD          # vt total columns
TQ = 4
H = ET // TQ
WARM_MM = 6

# node chunks: (ring, [blocks], square engine 'V'/'S', est arrival us)
CHUNKS = [
    ("G", [0], "S", 6.6),           # GE: eye + block 0
    ("G", [1, 2], "V", 11.0),       # GA
    ("G", [3, 4], "V", 13.0),       # GB
    ("G", [5], "V", 14.0),          # GC
    ("S", [6, 7, 8], "S", 13.0),    # SA (per-block ACT squares)
    ("A", [9, 10, 11], "V", 14.0),  # XA
    ("S", [12], "S", 16.0),         # SD
    ("A", [13], "V", 16.0),         # XD
]
DOT_ORDER = [0, 1, 2, 3, 4, 5, 6, 7, 8, 9, 10, 11, 12, 13]
SSQ_ORDER = [0, 1, 2, 3, 4, 6, 7, 8, 5, 9, 10, 11, 12, 13]
ARRIVE = {}
for _ring, _bs, _sq, _t in CHUNKS:
    for _b in _bs:
        ARRIVE[_b] = _t
S_READY = 12.0
SQ_DONE = {0: 10.0, 1: 12.0, 2: 12.2, 3: 13.9, 4: 14.1, 5: 14.6,
           6: 13.9, 7: 14.7, 8: 15.4, 9: 15.0, 10: 15.3, 11: 15.6,
           12: 16.9, 13: 16.8}

F32 = mybir.dt.float32
BF16 = mybir.dt.bfloat16
BF = ml_dtypes.bfloat16
ARSQRT = mybir.ActivationFunctionType.Abs_reciprocal_sqrt
SQUARE = mybir.ActivationFunctionType.Square
COPY = mybir.ActivationFunctionType.Copy

_cache = {}


def _build():
    nc = bacc.Bacc(
        "TRN2",
        target_bir_lowering=False,
        debug=False,
        enable_asserts=True,
        num_devices=N_CORES,
    )
    tgt = nc.dram_tensor("target", [E, D], BF16, kind="ExternalInput").ap()
    vt = nc.dram_tensor("vt", [D, NC2], BF16, kind="ExternalInput").ap()
    out = nc.dram_tensor("out", [NG * G], F32, kind="ExternalOutput").ap()

    with tile.TileContext(nc) as tc, ExitStack() as ctx:
        tpool = ctx.enter_context(tc.tile_pool(name="tgt", bufs=1))
        vpool = ctx.enter_context(tc.tile_pool(name="v", bufs=1))
        spool = ctx.enter_context(tc.tile_pool(name="small", bufs=1))
        scr = ctx.enter_context(tc.tile_pool(name="scr", bufs=1))
        ps_w = ctx.enter_context(tc.tile_pool(name="psw", bufs=1, space="PSUM"))
        ps_sr = ctx.enter_context(tc.tile_pool(name="pssr", bufs=1, space="PSUM"))
        ps_c0 = ctx.enter_context(tc.tile_pool(name="psc0", bufs=1, space="PSUM"))
        ps_c1 = ctx.enter_context(tc.tile_pool(name="psc1", bufs=1, space="PSUM"))
        ps_da = ctx.enter_context(tc.tile_pool(name="psda", bufs=1, space="PSUM"))
        ps_db = ctx.enter_context(tc.tile_pool(name="psdb", bufs=1, space="PSUM"))
        ps_qa = ctx.enter_context(tc.tile_pool(name="psqa", bufs=1, space="PSUM"))
        ps_qb = ctx.enter_context(tc.tile_pool(name="psqb", bufs=1, space="PSUM"))

        tgt_sb = tpool.tile([128, ET, D], BF16, name="tgt_sb")
        tsq = scr.tile([128, ET, D], BF16, name="tsq")
        vt_sb = vpool.tile([128, A, NC2], BF16, name="vt_sb")
        vsq = vpool.tile([128, A, NPAD], BF16, name="vsq")

        ssq_t = spool.tile([128, ET], F32, name="ssq_t")
        winv = spool.tile([128, ET], BF16, name="winv")
        s_bf = spool.tile([1, D], BF16, name="s_bf")
        one_bf = spool.tile([1, 1], BF16, name="one_bf")
        s_colbf = spool.tile([128, A], BF16, name="s_colbf")
        dotw = spool.tile([128, A, EYC], BF16, name="dotw")
        warm_w = spool.tile([128, 1], BF16, name="warm_w")
        warm_x = spool.tile([128, G], BF16, name="warm_x")
        act_d = spool.tile([1, 1], F32, name="act_d")
        act_s = spool.tile([1, 1], F32, name="act_s")
        isv = [
            spool.tile([NP, G], F32, name="isva"),
            spool.tile([NP, G], F32, name="isvb"),
        ]
        res = [
            spool.tile([NP, G], F32, name="resa"),
            spool.tile([NP, G], F32, name="resb"),
        ]

        warm_ps = ps_w.tile([1, G], F32, name="warm_ps")
        srow_ps = ps_sr.tile([1, D], F32, name="srow_ps")
        scol_ps = [
            ps_c0.tile([128, 1], F32, name="scol0"),
            ps_c1.tile([128, 1], F32, name="scol1"),
        ]
        dot_ps = [
            ps_da.tile([NP, G], F32, name="dot_psa"),
            ps_db.tile([NP, G], F32, name="dot_psb"),
        ]
        sq_ps = [
            ps_qa.tile([NP, G], F32, name="sq_psa"),
            ps_qb.tile([NP, G], F32, name="sq_psb"),
        ]

        tgt_v = tgt.rearrange("(p j) d -> p j d", j=ET)
        vt_v = vt.rearrange("(a p) n -> p a n", p=128)
        out_v = out.rearrange("(g f) -> g f", f=G)
        eye2d = vt_sb[:, 0, 0:EYC]

        def W(us):
            return tc.tile_wait_until(us / 1000.0)

        def vcols(b0, b1):
            return slice(EYC + b0 * G, EYC + b1 * G)

        ring_eng = {"S": nc.sync, "A": nc.scalar, "G": nc.gpsimd}

        # ---- HWDGE input DMA issues first (tgt heads both rings)
        for q in range(2):
            nc.sync.dma_start(
                tgt_sb[:, q * H : (q + 1) * H, :], tgt_v[:, q * H : (q + 1) * H, :]
            )
        for q in range(2, 4):
            nc.scalar.dma_start(
                tgt_sb[:, q * H : (q + 1) * H, :], tgt_v[:, q * H : (q + 1) * H, :]
            )
        nc.vector.memset(act_d[:], 1.0)
        nc.scalar.activation(act_s[:], act_d[:], ARSQRT)  # pins the table load
        # SP/ACT node chunks (queue behind tgt on their rings)
        for ring, bs, _sq, _t in CHUNKS:
            if ring == "G":
                continue
            sl = slice(EYC + bs[0] * G, EYC + (bs[-1] + 1) * G)
            ring_eng[ring].dma_start(vt_sb[:, :, sl], vt_v[:, :, sl])
        # GP ring: eye+b0 now; bulk chunks AFTER the gpsimd phase-A square
        # (keeps early HBM bandwidth for tgt)
        nc.gpsimd.dma_start(vt_sb[:, :, 0 : EYC + G], vt_v[:, :, 0 : EYC + G])

        # ---- consts
        nc.vector.memset(warm_w[:], 1.0)
        nc.vector.memset(warm_x[:], 0.0)
        nc.vector.memset(one_bf[:], 1.0)

        # ---- PE prewarm
        for _ in range(WARM_MM):
            nc.tensor.matmul(warm_ps[:], warm_w[:], warm_x[:], start=True, stop=True)

        # ---- phase A ssq: q0/q1 DVE, q2 GP square + DVE reduce, q3 ACT
        for q in (0, 1):
            sl = slice(q * H, (q + 1) * H)
            nc.vector.tensor_mul(tsq[:, sl, :], tgt_sb[:, sl, :], tgt_sb[:, sl, :])
            with tc.high_priority():
                nc.vector.tensor_reduce(
                    ssq_t[:, sl], tsq[:, sl, :],
                    axis=mybir.AxisListType.X, op=mybir.AluOpType.add,
                )
                nc.scalar.activation(winv[:, sl], ssq_t[:, sl], ARSQRT)
        sl2 = slice(2 * H, 3 * H)
        nc.gpsimd.tensor_mul(tsq[:, sl2, :], tgt_sb[:, sl2, :], tgt_sb[:, sl2, :])
        # GP bulk chunks issue after the gpsimd square
        for ring, bs, _sq, _t in CHUNKS[1:]:
            if ring != "G":
                continue
            sl = slice(EYC + bs[0] * G, EYC + (bs[-1] + 1) * G)
            nc.gpsimd.dma_start(vt_sb[:, :, sl], vt_v[:, :, sl])
        with tc.high_priority():
            nc.vector.tensor_reduce(
                ssq_t[:, sl2], tsq[:, sl2, :],
                axis=mybir.AxisListType.X, op=mybir.AluOpType.add,
            )
            nc.scalar.activation(winv[:, sl2], ssq_t[:, sl2], ARSQRT)
        with tc.high_priority():
            for j in range(3 * H, ET):
                nc.scalar.activation(
                    tsq[:, j, :], tgt_sb[:, j, :], SQUARE,
                    accum_out=ssq_t[:, j : j + 1],
                )
            sl3 = slice(3 * H, ET)
            nc.scalar.activation(winv[:, sl3], ssq_t[:, sl3], ARSQRT)

        # ---- phase A s-row matmuls, emitted in readiness order
        JORDER = list(range(0, 2 * H)) + list(range(3 * H, ET)) + list(range(2 * H, 3 * H))
        for i, j in enumerate(JORDER):
            nc.tensor.matmul(
                srow_ps[:],
                winv[:, j : j + 1],
                tgt_sb[:, j, :],
                start=(i == 0),
                stop=(i == ET - 1),
            )
        # s_bf split per half so each colmm starts early
        for a in range(A):
            with tc.high_priority():
                nc.scalar.activation(
                    s_bf[:, a * 128 : (a + 1) * 128],
                    srow_ps[:, a * 128 : (a + 1) * 128],
                    COPY,
                )
            nc.tensor.matmul(
                scol_ps[a][:],
                s_bf[:, a * 128 : (a + 1) * 128],
                one_bf[:],
                start=True,
                stop=True,
            )
            with tc.high_priority():
                nc.scalar.activation(
                    s_colbf[:, a : a + 1], scol_ps[a][:], COPY, scale=-1.0
                )
                nc.vector.tensor_mul(
                    dotw[:, a],
                    eye2d,
                    s_colbf[:, a : a + 1].broadcast_to([128, EYC]),
                )

        # ---- node squares
        for ring, bs, sqe, t_arr in CHUNKS:
            if sqe == "S":
                for b in bs:
                    with W(t_arr + 0.2):
                        nc.scalar.activation(
                            vsq[:, :, b * G : (b + 1) * G],
                            vt_sb[:, :, vcols(b, b + 1)],
                            SQUARE,
                        )
            else:
                with W(t_arr + 0.2):
                    nc.vector.tensor_mul(
                        vsq[:, :, bs[0] * G : (bs[-1] + 1) * G],
                        vt_sb[:, :, vcols(bs[0], bs[-1] + 1)],
                        vt_sb[:, :, vcols(bs[0], bs[-1] + 1)],
                    )

        # ---- PE node matmuls
        def pair_of(b):
            return (0, b) if b < NP else (1, b - NP)

        def emit_mms(order, ps, lhs_for, t_of):
            first_seen = {0: True, 1: True}
            remaining = {0: sum(1 for b in order if b < NP),
                         1: sum(1 for b in order if b >= NP)}
            for b in order:
                p, r = pair_of(b)
                remaining[p] -= 1
                with W(t_of(b)):
                    for a in range(A):
                        nc.tensor.matmul(
                            ps[p][:],
                            lhs_for(a, r),
                            (vsq[:, a, b * G : (b + 1) * G]
                             if ps is sq_ps
                             else vt_sb[:, a, vcols(b, b + 1)]),
                            start=(first_seen[p] and a == 0),
                            stop=(remaining[p] == 0 and a == 1),
                        )
                first_seen[p] = False

        emit_mms(
            DOT_ORDER, dot_ps,
            lambda a, r: dotw[:, a, r * NP : (r + 1) * NP],
            lambda b: max(S_READY, ARRIVE[b] + 0.2),
        )
        emit_mms(
            SSQ_ORDER, sq_ps,
            lambda a, r: eye2d[:, r * NP : (r + 1) * NP],
            lambda b: SQ_DONE[b],
        )
        # tails
        for p, t_tail in ((0, 15.8), (1, 17.4)):
            with W(t_tail):
                nc.scalar.activation(isv[p][:], sq_ps[p][:], ARSQRT)
                nc.vector.tensor_mul(res[p][:], dot_ps[p][:], isv[p][:])
                nc.sync.dma_start(out_v[p * NP : (p + 1) * NP, :], res[p][:])

    nc.compile()
    return nc


def _get_nc():
    if "nc" not in _cache:
        _cache["nc"] = _build()
    return _cache["nc"]


def _host_inputs(target, node_emb):
    tgt_bf = np.ascontiguousarray(np.asarray(target, dtype=np.float32)).astype(BF)
    node_emb = np.asarray(node_emb, dtype=np.float32)

    eye = np.zeros((128, EYC), dtype=BF)
    for r in range(NP):
        eye[:, r * NP + r] = 1.0

    in_maps = []
    for c in range(N_CORES):
        shard = np.empty((NPAD, D), dtype=np.float32)
        shard[:NPC] = node_emb[c * NPC : (c + 1) * NPC]
        shard[NPC:] = node_emb[: NPAD - NPC]  # pad with real rows (no 0-norm)
        vtp = np.empty((D, NC2), dtype=BF)
        vtp[:128, 0:EYC] = eye
        vtp[128:, 0:EYC] = 0
        vtp[:, EYC:] = shard.T.astype(BF)
        in_maps.append(
            {"target": tgt_bf, "vt": np.ascontiguousarray(vtp)}
        )
    return in_maps


def run(pred, target, node_emb, trace=False, **trace_kwargs):
    """Returns (full_output [50000] f32, BassKernelResults)."""
    nc = _get_nc()
    in_maps = _host_inputs(target, node_emb)
    res = bass_utils.run_bass_kernel_spmd(
        nc, in_maps, list(range(N_CORES)), trace=trace, **trace_kwargs
    )
    parts = [res.results[c]["out"][:NPC] for c in range(N_CORES)]
    return np.concatenate(parts).astype(np.float32), res


def kernel(pred, target, node_emb):
    out, _ = run(pred, target, node_emb)
    return out


# revision 22
# speedup vs baseline: 1.0011x; 1.0011x over previous
"""ContrastiveDist kernel for TRN2 (8 NeuronCores, SPMD) -- v4.5.

out[n] = sum_e -(t_e . v_n) / (||t_e|| * ||v_n|| + eps)
       = (s . v_n) / ||v_n||      with s = -sum_e t_e / ||t_e||
(eps shifts the result by ~4e-11 relative -- dropped.)

Schedule design (from the v3 / v4.0-v4.4 traces):
 * THREE DMA queues: SP HWDGE (nc.sync), ACT HWDGE (nc.scalar), GPSIMD
   SWDGE (nc.gpsimd); ~285 GB/s aggregate HBM-bound, round-robin per
   packet.  tgt quarters head both HWDGE rings.  ALL DMA issues carry
   high_priority: they are data-ready at sim t=0, so the static
   scheduler always places them ahead of (mispredicted) compute --
   v4.4's XD issue sat behind phase-A ACT work until 19.6us.
 * eye and dotw are built ON DEVICE with strided memsets/copies
   (diagonal of a [128, 49] tile = stride-8 free-axis slice) -- no eye
   DMA, no eye dependency in the s chain, and the GP ring's first chunk
   is a clean single node block.
 * target entity-major [128e, 16, 256d], 4 quarters: DVE square+reduce
   for q0/q1, GPSIMD square -> DVE reduce for q2/q3.  ACT
   Abs_reciprocal_sqrt emits winv in BF16 directly; the s sign folds
   into ACT Copy(scale=-1) column copies; s_bf copies split per d-half.
 * GP ring: first block immediately, bulk chunks issued after the
   gpsimd phase-A square so tgt keeps the early bandwidth.
 * PE prewarm + filler matmuls bridge the phase-A gaps so the HAM clock
   gate keeps the PE at 2.4 GHz for the node matmuls.
 * psum pairs: blocks 0-6 / 7-13; block-diag lhsT routes block b to its
   psum row; tails are ACT arsqrt [7,448] + one DVE mul; outs on SP.
 * fused DVE tensor_tensor_reduce is NOT used anywhere (locks up HW).
"""

import numpy as np
import ml_dtypes
from contextlib import ExitStack

import concourse.bacc as bacc
import concourse.bass as bass
import concourse.mybir as mybir
import concourse.tile as tile
from concourse import bass_utils

E, D = 2048, 256
N_FULL = 50000
N_CORES = 8
NPC = N_FULL // N_CORES
G = 448
NG = 14
NPAD = G * NG
NP = 7
A = 2
ET = E // 128
EYC = NP * NP            # eye tile columns (on-device)
TQ = 4
H = ET // TQ
WARM_MM = 6

# node chunks: (ring, [blocks], square engine 'V'/'S', est arrival us,
#               issue early?)
CHUNKS = [
    ("G", [0], "S", 10.7, True),            # GE
    ("G", [1, 2], "V", 14.2, True),         # GA
    ("G", [3, 4], "V", 19.2, False),        # GB (issued after gp TTq2)
    ("G", [5], "V", 22.6, False),           # GC
    ("S", [6, 7, 8], "S", 20.0, True),      # SA
    ("A", [9, 10, 11], "V", 20.0, True),    # XA
    ("S", [12], "S", 23.0, True),           # SD
    ("A", [13], "V", 23.0, True),           # XD
]
DOT_ORDER = [0, 1, 2, 3, 4, 6, 7, 8, 9, 10, 11, 5, 12, 13]
SSQ_ORDER = [0, 1, 2, 3, 4, 6, 7, 8, 9, 10, 11, 5, 12, 13]
ARRIVE = {}
for _ring, _bs, _sq, _t, _e in CHUNKS:
    for _b in _bs:
        ARRIVE[_b] = _t
S_READY = 16.5

F32 = mybir.dt.float32
BF16 = mybir.dt.bfloat16
BF = ml_dtypes.bfloat16
ARSQRT = mybir.ActivationFunctionType.Abs_reciprocal_sqrt
SQUARE = mybir.ActivationFunctionType.Square
COPY = mybir.ActivationFunctionType.Copy

_cache = {}


def _build():
    nc = bacc.Bacc(
        "TRN2",
        target_bir_lowering=False,
        debug=False,
        enable_asserts=True,
        num_devices=N_CORES,
    )
    tgt = nc.dram_tensor("target", [E, D], BF16, kind="ExternalInput").ap()
    vt = nc.dram_tensor("vt", [D, NPAD], BF16, kind="ExternalInput").ap()
    out = nc.dram_tensor("out", [NG * G], F32, kind="ExternalOutput").ap()

    with tile.TileContext(nc) as tc, ExitStack() as ctx:
        tpool = ctx.enter_context(tc.tile_pool(name="tgt", bufs=1))
        vpool = ctx.enter_context(tc.tile_pool(name="v", bufs=1))
        spool = ctx.enter_context(tc.tile_pool(name="small", bufs=1))
        scr = ctx.enter_context(tc.tile_pool(name="scr", bufs=1))
        ps_w = ctx.enter_context(tc.tile_pool(name="psw", bufs=1, space="PSUM"))
        ps_sr = ctx.enter_context(tc.tile_pool(name="pssr", bufs=1, space="PSUM"))
        ps_c0 = ctx.enter_context(tc.tile_pool(name="psc0", bufs=1, space="PSUM"))
        ps_c1 = ctx.enter_context(tc.tile_pool(name="psc1", bufs=1, space="PSUM"))
        ps_da = ctx.enter_context(tc.tile_pool(name="psda", bufs=1, space="PSUM"))
        ps_db = ctx.enter_context(tc.tile_pool(name="psdb", bufs=1, space="PSUM"))
        ps_qa = ctx.enter_context(tc.tile_pool(name="psqa", bufs=1, space="PSUM"))
        ps_qb = ctx.enter_context(tc.tile_pool(name="psqb", bufs=1, space="PSUM"))

        tgt_sb = tpool.tile([128, ET, D], BF16, name="tgt_sb")
        tsq = scr.tile([128, ET, D], BF16, name="tsq")
        vt_sb = vpool.tile([128, A, NPAD], BF16, name="vt_sb")
        vsq = vpool.tile([128, A, NPAD], BF16, name="vsq")

        ssq_t = spool.tile([128, ET], F32, name="ssq_t")
        winv = spool.tile([128, ET], BF16, name="winv")
        s_bf = spool.tile([1, D], BF16, name="s_bf")
        one_bf = spool.tile([1, 1], BF16, name="one_bf")
        s_colbf = spool.tile([128, A], BF16, name="s_colbf")
        eye2d = spool.tile([128, EYC], BF16, name="eye2d")
        dotw = spool.tile([128, A, EYC], BF16, name="dotw")
        warm_w = spool.tile([128, 1], BF16, name="warm_w")
        warm_x = spool.tile([128, G], BF16, name="warm_x")
        act_d = spool.tile([1, 1], F32, name="act_d")
        act_s = spool.tile([1, 1], F32, name="act_s")
        isv = [
            spool.tile([NP, G], F32, name="isva"),
            spool.tile([NP, G], F32, name="isvb"),
        ]
        res = [
            spool.tile([NP, G], F32, name="resa"),
            spool.tile([NP, G], F32, name="resb"),
        ]

        warm_ps = ps_w.tile([1, G], F32, name="warm_ps")
        srow_ps = ps_sr.tile([1, D], F32, name="srow_ps")
        scol_ps = [
            ps_c0.tile([128, 1], F32, name="scol0"),
            ps_c1.tile([128, 1], F32, name="scol1"),
        ]
        dot_ps = [
            ps_da.tile([NP, G], F32, name="dot_psa"),
            ps_db.tile([NP, G], F32, name="dot_psb"),
        ]
        sq_ps = [
            ps_qa.tile([NP, G], F32, name="sq_psa"),
            ps_qb.tile([NP, G], F32, name="sq_psb"),
        ]

        tgt_v = tgt.rearrange("(p j) d -> p j d", j=ET)
        vt_v = vt.rearrange("(a p) n -> p a n", p=128)
        out_v = out.rearrange("(g f) -> g f", f=G)

        def W(us):
            return tc.tile_wait_until(us / 1000.0)

        def bcols(b0, b1):
            return slice(b0 * G, b1 * G)

        ring_eng = {"S": nc.sync, "A": nc.scalar, "G": nc.gpsimd}

        # ---- DMA issues (high_priority: data-ready at t=0, always first)
        with tc.high_priority():
            for q in range(2):
                nc.sync.dma_start(
                    tgt_sb[:, q * H : (q + 1) * H, :],
                    tgt_v[:, q * H : (q + 1) * H, :],
                )
            for q in range(2, 4):
                nc.scalar.dma_start(
                    tgt_sb[:, q * H : (q + 1) * H, :],
                    tgt_v[:, q * H : (q + 1) * H, :],
                )
            for ring, bs, _sq, _t, early in CHUNKS:
                if not early:
                    continue
                sl = bcols(bs[0], bs[-1] + 1)
                ring_eng[ring].dma_start(vt_sb[:, :, sl], vt_v[:, :, sl])
        nc.vector.memset(act_d[:], 1.0)
        nc.scalar.activation(act_s[:], act_d[:], ARSQRT)  # pins the table load

        # ---- consts / on-device eye + dotw skeleton
        nc.vector.memset(warm_w[:], 1.0)
        nc.vector.memset(warm_x[:], 0.0)
        nc.vector.memset(one_bf[:], 1.0)
        nc.vector.memset(eye2d[:], 0.0)
        nc.vector.memset(eye2d[:, 0 : EYC : NP + 1], 1.0)
        nc.vector.memset(dotw[:], 0.0)

        # ---- PE prewarm + fillers through the phase-A window
        for _ in range(WARM_MM):
            nc.tensor.matmul(warm_ps[:], warm_w[:], warm_x[:], start=True, stop=True)
        for t_fill in (10.5, 11.5, 12.5, 13.5):
            with W(t_fill):
                nc.tensor.matmul(
                    warm_ps[:], warm_w[:], warm_x[:], start=True, stop=True
                )

        # ---- phase A ssq: q0/q1 DVE, q2/q3 GP square -> DVE reduce
        for q in (0, 1):
            sl = slice(q * H, (q + 1) * H)
            nc.vector.tensor_mul(tsq[:, sl, :], tgt_sb[:, sl, :], tgt_sb[:, sl, :])
            with tc.high_priority():
                nc.vector.tensor_reduce(
                    ssq_t[:, sl], tsq[:, sl, :],
                    axis=mybir.AxisListType.X, op=mybir.AluOpType.add,
                )
                nc.scalar.activation(winv[:, sl], ssq_t[:, sl], ARSQRT)
        for q in (2, 3):
            sl = slice(q * H, (q + 1) * H)
            nc.gpsimd.tensor_mul(tsq[:, sl, :], tgt_sb[:, sl, :], tgt_sb[:, sl, :])
            if q == 2:
                # GP bulk chunks issue after the first gpsimd square
                for ring, bs, _sq, _t, early in CHUNKS:
                    if early or ring != "G":
                        continue
                    slc = bcols(bs[0], bs[-1] + 1)
                    nc.gpsimd.dma_start(vt_sb[:, :, slc], vt_v[:, :, slc])
            with tc.high_priority():
                nc.vector.tensor_reduce(
                    ssq_t[:, sl], tsq[:, sl, :],
                    axis=mybir.AxisListType.X, op=mybir.AluOpType.add,
                )
                nc.scalar.activation(winv[:, sl], ssq_t[:, sl], ARSQRT)

        # ---- phase A s-row matmuls
        for j in range(ET):
            nc.tensor.matmul(
                srow_ps[:],
                winv[:, j : j + 1],
                tgt_sb[:, j, :],
                start=(j == 0),
                stop=(j == ET - 1),
            )
        for a in range(A):
            with tc.high_priority():
                nc.scalar.activation(
                    s_bf[:, a * 128 : (a + 1) * 128],
                    srow_ps[:, a * 128 : (a + 1) * 128],
                    COPY,
                )
            nc.tensor.matmul(
                scol_ps[a][:],
                s_bf[:, a * 128 : (a + 1) * 128],
                one_bf[:],
                start=True,
                stop=True,
            )
            with tc.high_priority():
                nc.scalar.activation(
                    s_colbf[:, a : a + 1], scol_ps[a][:], COPY, scale=-1.0
                )
                # dotw diagonal = -s (stride NP+1 hits [r, r])
                nc.vector.tensor_copy(
                    dotw[:, a, 0 : EYC : NP + 1],
                    s_colbf[:, a : a + 1].broadcast_to([128, NP]),
                )

        # ---- node squares
        for ring, bs, sqe, t_arr, _e in CHUNKS:
            if sqe == "S":
                for b in bs:
                    with W(t_arr + 0.2):
                        nc.scalar.activation(
                            vsq[:, :, b * G : (b + 1) * G],
                            vt_sb[:, :, bcols(b, b + 1)],
                            SQUARE,
                        )
            else:
                with W(t_arr + 0.2):
                    nc.vector.tensor_mul(
                        vsq[:, :, bs[0] * G : (bs[-1] + 1) * G],
                        vt_sb[:, :, bcols(bs[0], bs[-1] + 1)],
                        vt_sb[:, :, bcols(bs[0], bs[-1] + 1)],
                    )

        # ---- PE node matmuls
        def pair_of(b):
            return (0, b) if b < NP else (1, b - NP)

        def emit_mms(order, ps, lhs_for, t_of):
            first_seen = {0: True, 1: True}
            remaining = {0: sum(1 for b in order if b < NP),
                         1: sum(1 for b in order if b >= NP)}
            for b in order:
                p, r = pair_of(b)
                remaining[p] -= 1
                with W(t_of(b)):
                    for a in range(A):
                        nc.tensor.matmul(
                            ps[p][:],
                            lhs_for(a, r),
                            (vsq[:, a, b * G : (b + 1) * G]
                             if ps is sq_ps
                             else vt_sb[:, a, bcols(b, b + 1)]),
                            start=(first_seen[p] and a == 0),
                            stop=(remaining[p] == 0 and a == 1),
                        )
                first_seen[p] = False

        emit_mms(
            DOT_ORDER, dot_ps,
            lambda a, r: dotw[:, a, r * NP : (r + 1) * NP],
            lambda b: max(S_READY, ARRIVE[b] + 0.2),
        )
        emit_mms(
            SSQ_ORDER, sq_ps,
            lambda a, r: eye2d[:, r * NP : (r + 1) * NP],
            lambda b: ARRIVE[b] + 0.7,
        )
        # tails
        for p, t_tail in ((0, 23.3), (1, 23.8)):
            with W(t_tail):
                nc.scalar.activation(isv[p][:], sq_ps[p][:], ARSQRT)
                nc.vector.tensor_mul(res[p][:], dot_ps[p][:], isv[p][:])
                nc.sync.dma_start(out_v[p * NP : (p + 1) * NP, :], res[p][:])

    nc.compile()
    return nc


def _get_nc():
    if "nc" not in _cache:
        _cache["nc"] = _build()
    return _cache["nc"]


def _host_inputs(target, node_emb):
    tgt_bf = np.ascontiguousarray(np.asarray(target, dtype=np.float32)).astype(BF)
    node_emb = np.asarray(node_emb, dtype=np.float32)

    in_maps = []
    for c in range(N_CORES):
        shard = np.empty((NPAD, D), dtype=np.float32)
        shard[:NPC] = node_emb[c * NPC : (c + 1) * NPC]
        shard[NPC:] = node_emb[: NPAD - NPC]  # pad with real rows (no 0-norm)
        vtp = np.ascontiguousarray(shard.T.astype(BF))
        in_maps.append({"target": tgt_bf, "vt": vtp})
    return in_maps


def run(pred, target, node_emb, trace=False, **trace_kwargs):
    """Returns (full_output [50000] f32, BassKernelResults)."""
    nc = _get_nc()
    in_maps = _host_inputs(target, node_emb)
    res = bass_utils.run_bass_kernel_spmd(
        nc, in_maps, list(range(N_CORES)), trace=trace, **trace_kwargs
    )
    parts = [res.results[c]["out"][:NPC] for c in range(N_CORES)]
    return np.concatenate(parts).astype(np.float32), res


def kernel(pred, target, node_emb):
    out, _ = run(pred, target, node_emb)
    return out


# revision 23
# speedup vs baseline: 1.0033x; 1.0022x over previous
"""ContrastiveDist kernel for TRN2 (8 NeuronCores, SPMD) -- v4.5.

out[n] = sum_e -(t_e . v_n) / (||t_e|| * ||v_n|| + eps)
       = (s . v_n) / ||v_n||      with s = -sum_e t_e / ||t_e||
(eps shifts the result by ~4e-11 relative -- dropped.)

Schedule design (from the v3 / v4.0-v4.4 traces):
 * THREE DMA queues: SP HWDGE (nc.sync), ACT HWDGE (nc.scalar), GPSIMD
   SWDGE (nc.gpsimd); ~285 GB/s aggregate HBM-bound, round-robin per
   packet.  tgt quarters head both HWDGE rings.  ALL DMA issues carry
   high_priority: they are data-ready at sim t=0, so the static
   scheduler always places them ahead of (mispredicted) compute --
   v4.4's XD issue sat behind phase-A ACT work until 19.6us.
 * eye and dotw are built ON DEVICE with strided memsets/copies
   (diagonal of a [128, 49] tile = stride-8 free-axis slice) -- no eye
   DMA, no eye dependency in the s chain, and the GP ring's first chunk
   is a clean single node block.
 * target entity-major [128e, 16, 256d], 4 quarters: DVE square+reduce
   for q0/q1, GPSIMD square -> DVE reduce for q2/q3.  ACT
   Abs_reciprocal_sqrt emits winv in BF16 directly; the s sign folds
   into ACT Copy(scale=-1) column copies; s_bf copies split per d-half.
 * GP ring: first block immediately, bulk chunks issued after the
   gpsimd phase-A square so tgt keeps the early bandwidth.
 * PE prewarm + filler matmuls bridge the phase-A gaps so the HAM clock
   gate keeps the PE at 2.4 GHz for the node matmuls.
 * psum pairs: blocks 0-6 / 7-13; block-diag lhsT routes block b to its
   psum row; tails are ACT arsqrt [7,448] + one DVE mul; outs on SP.
 * fused DVE tensor_tensor_reduce is NOT used anywhere (locks up HW).
"""

import numpy as np
import ml_dtypes
from contextlib import ExitStack

import concourse.bacc as bacc
import concourse.bass as bass
import concourse.mybir as mybir
import concourse.tile as tile
from concourse import bass_utils

E, D = 2048, 256
N_FULL = 50000
N_CORES = 8
NPC = N_FULL // N_CORES
G = 448
NG = 14
NPAD = G * NG
NP = 7
A = 2
ET = E // 128
EYC = NP * NP            # eye tile columns (on-device)
TQ = 4
H = ET // TQ
WARM_MM = 6

# node chunks: (ring, [blocks], square engine 'V'/'S', est arrival us,
#               issue early?)
CHUNKS = [
    ("G", [0], "S", 10.7, True),            # GE
    ("G", [1, 2], "V", 14.2, True),         # GA
    ("G", [3, 4], "V", 19.2, False),        # GB (issued after gp TTq2)
    ("G", [5], "V", 22.6, False),           # GC
    ("S", [6, 7, 8], "S", 20.0, True),      # SA
    ("A", [9, 10, 11], "V", 20.0, True),    # XA
    ("S", [12], "S", 23.0, True),           # SD
    ("A", [13], "V", 23.0, True),           # XD
]
DOT_ORDER = [0, 1, 2, 3, 4, 6, 7, 8, 9, 10, 11, 5, 12, 13]
SSQ_ORDER = [0, 1, 2, 3, 4, 6, 7, 8, 9, 10, 11, 5, 12, 13]
ARRIVE = {}
for _ring, _bs, _sq, _t, _e in CHUNKS:
    for _b in _bs:
        ARRIVE[_b] = _t
S_READY = 18.2

F32 = mybir.dt.float32
BF16 = mybir.dt.bfloat16
BF = ml_dtypes.bfloat16
ARSQRT = mybir.ActivationFunctionType.Abs_reciprocal_sqrt
SQUARE = mybir.ActivationFunctionType.Square
COPY = mybir.ActivationFunctionType.Copy

_cache = {}


def _build():
    nc = bacc.Bacc(
        "TRN2",
        target_bir_lowering=False,
        debug=False,
        enable_asserts=True,
        num_devices=N_CORES,
    )
    tgt = nc.dram_tensor("target", [E, D], BF16, kind="ExternalInput").ap()
    vt = nc.dram_tensor("vt", [D, NPAD], BF16, kind="ExternalInput").ap()
    out = nc.dram_tensor("out", [NG * G], F32, kind="ExternalOutput").ap()

    with tile.TileContext(nc) as tc, ExitStack() as ctx:
        tpool = ctx.enter_context(tc.tile_pool(name="tgt", bufs=1))
        vpool = ctx.enter_context(tc.tile_pool(name="v", bufs=1))
        spool = ctx.enter_context(tc.tile_pool(name="small", bufs=1))
        scr = ctx.enter_context(tc.tile_pool(name="scr", bufs=1))
        ps_w = ctx.enter_context(tc.tile_pool(name="psw", bufs=1, space="PSUM"))
        ps_sr = ctx.enter_context(tc.tile_pool(name="pssr", bufs=1, space="PSUM"))
        ps_c0 = ctx.enter_context(tc.tile_pool(name="psc0", bufs=1, space="PSUM"))
        ps_c1 = ctx.enter_context(tc.tile_pool(name="psc1", bufs=1, space="PSUM"))
        ps_da = ctx.enter_context(tc.tile_pool(name="psda", bufs=1, space="PSUM"))
        ps_db = ctx.enter_context(tc.tile_pool(name="psdb", bufs=1, space="PSUM"))
        ps_qa = ctx.enter_context(tc.tile_pool(name="psqa", bufs=1, space="PSUM"))
        ps_qb = ctx.enter_context(tc.tile_pool(name="psqb", bufs=1, space="PSUM"))

        tgt_sb = tpool.tile([128, ET, D], BF16, name="tgt_sb")
        tsq = scr.tile([128, ET, D], BF16, name="tsq")
        vt_sb = vpool.tile([128, A, NPAD], BF16, name="vt_sb")
        vsq = vpool.tile([128, A, NPAD], BF16, name="vsq")

        ssq_t = spool.tile([128, ET], F32, name="ssq_t")
        winv = spool.tile([128, ET], BF16, name="winv")
        s_colbf = spool.tile([128, A], BF16, name="s_colbf")
        eye2d = spool.tile([128, EYC], BF16, name="eye2d")
        dotw = spool.tile([128, A, EYC], BF16, name="dotw")
        warm_w = spool.tile([128, 1], BF16, name="warm_w")
        warm_x = spool.tile([128, G], BF16, name="warm_x")
        act_d = spool.tile([1, 1], F32, name="act_d")
        act_s = spool.tile([1, 1], F32, name="act_s")
        isv = [
            spool.tile([NP, G], F32, name="isva"),
            spool.tile([NP, G], F32, name="isvb"),
        ]
        res = [
            spool.tile([NP, G], F32, name="resa"),
            spool.tile([NP, G], F32, name="resb"),
        ]

        warm_ps = ps_w.tile([1, G], F32, name="warm_ps")
        scol_ps = [
            ps_c0.tile([128, 1], F32, name="scol0"),
            ps_c1.tile([128, 1], F32, name="scol1"),
        ]
        dot_ps = [
            ps_da.tile([NP, G], F32, name="dot_psa"),
            ps_db.tile([NP, G], F32, name="dot_psb"),
        ]
        sq_ps = [
            ps_qa.tile([NP, G], F32, name="sq_psa"),
            ps_qb.tile([NP, G], F32, name="sq_psb"),
        ]

        tgt_v = tgt.rearrange("(p j) d -> p j d", j=ET)
        vt_v = vt.rearrange("(a p) n -> p a n", p=128)
        out_v = out.rearrange("(g f) -> g f", f=G)

        def W(us):
            return tc.tile_wait_until(us / 1000.0)

        def bcols(b0, b1):
            return slice(b0 * G, b1 * G)

        ring_eng = {"S": nc.sync, "A": nc.scalar, "G": nc.gpsimd}

        # ---- DMA issues (high_priority: data-ready at t=0, always first)
        with tc.high_priority():
            for q in range(2):
                nc.sync.dma_start(
                    tgt_sb[:, q * H : (q + 1) * H, :],
                    tgt_v[:, q * H : (q + 1) * H, :],
                )
            for q in range(2, 4):
                nc.scalar.dma_start(
                    tgt_sb[:, q * H : (q + 1) * H, :],
                    tgt_v[:, q * H : (q + 1) * H, :],
                )
            for ring, bs, _sq, _t, early in CHUNKS:
                if not early:
                    continue
                sl = bcols(bs[0], bs[-1] + 1)
                ring_eng[ring].dma_start(vt_sb[:, :, sl], vt_v[:, :, sl])
        nc.vector.memset(act_d[:], 1.0)
        nc.scalar.activation(act_s[:], act_d[:], ARSQRT)  # pins the table load

        # ---- consts / on-device eye + dotw skeleton
        nc.vector.memset(warm_w[:], 1.0)
        nc.vector.memset(warm_x[:], 0.0)
        nc.vector.memset(eye2d[:], 0.0)
        nc.vector.memset(eye2d[:, 0 : EYC : NP + 1], 1.0)
        nc.vector.memset(dotw[:], 0.0)

        # ---- PE prewarm + fillers through the phase-A window
        for _ in range(WARM_MM):
            nc.tensor.matmul(warm_ps[:], warm_w[:], warm_x[:], start=True, stop=True)

        # ---- phase A ssq: q0/q1 DVE sq+red, q2 GP sq -> DVE red,
        # q3 split: 2 tiles ACT square+accum, 2 tiles GP sq -> DVE red
        for q in (0, 1):
            sl = slice(q * H, (q + 1) * H)
            nc.vector.tensor_mul(tsq[:, sl, :], tgt_sb[:, sl, :], tgt_sb[:, sl, :])
            with tc.high_priority():
                nc.vector.tensor_reduce(
                    ssq_t[:, sl], tsq[:, sl, :],
                    axis=mybir.AxisListType.X, op=mybir.AluOpType.add,
                )
                nc.scalar.activation(winv[:, sl], ssq_t[:, sl], ARSQRT)
        sl = slice(2 * H, 3 * H)
        nc.gpsimd.tensor_mul(tsq[:, sl, :], tgt_sb[:, sl, :], tgt_sb[:, sl, :])
        # GP bulk chunks issue after the first gpsimd square
        for ring, bs, _sq, _t, early in CHUNKS:
            if early or ring != "G":
                continue
            slc = bcols(bs[0], bs[-1] + 1)
            nc.gpsimd.dma_start(vt_sb[:, :, slc], vt_v[:, :, slc])
        with tc.high_priority():
            nc.vector.tensor_reduce(
                ssq_t[:, sl], tsq[:, sl, :],
                axis=mybir.AxisListType.X, op=mybir.AluOpType.add,
            )
            nc.scalar.activation(winv[:, sl], ssq_t[:, sl], ARSQRT)
        sl = slice(3 * H, 3 * H + 2)
        nc.gpsimd.tensor_mul(tsq[:, sl, :], tgt_sb[:, sl, :], tgt_sb[:, sl, :])
        with tc.high_priority():
            nc.vector.tensor_reduce(
                ssq_t[:, sl], tsq[:, sl, :],
                axis=mybir.AxisListType.X, op=mybir.AluOpType.add,
            )
            for j in range(3 * H + 2, ET):
                nc.scalar.activation(
                    tsq[:, j, :], tgt_sb[:, j, :], SQUARE,
                    accum_out=ssq_t[:, j : j + 1],
                )
            sl3 = slice(3 * H, ET)
            nc.scalar.activation(winv[:, sl3], ssq_t[:, sl3], ARSQRT)

        # ---- phase A s-column matmuls (v3 scheme: 128-col weight loads
        # pipeline under the 1-col streams; output is s_col psum direct)
        for j in range(ET):
            for a in range(A):
                nc.tensor.matmul(
                    scol_ps[a][:],
                    tgt_sb[:, j, a * 128 : (a + 1) * 128],
                    winv[:, j : j + 1],
                    start=(j == 0),
                    stop=(j == ET - 1),
                )
        for a in range(A):
            with tc.high_priority():
                nc.scalar.activation(
                    s_colbf[:, a : a + 1], scol_ps[a][:], COPY, scale=-1.0
                )
                # dotw diagonal = -s (stride NP+1 hits [r, r])
                nc.vector.tensor_copy(
                    dotw[:, a, 0 : EYC : NP + 1],
                    s_colbf[:, a : a + 1].broadcast_to([128, NP]),
                )

        # ---- node squares
        for ring, bs, sqe, t_arr, _e in CHUNKS:
            if sqe == "S":
                for b in bs:
                    with W(t_arr + 0.2):
                        nc.scalar.activation(
                            vsq[:, :, b * G : (b + 1) * G],
                            vt_sb[:, :, bcols(b, b + 1)],
                            SQUARE,
                        )
            else:
                with W(t_arr + 0.2):
                    nc.vector.tensor_mul(
                        vsq[:, :, bs[0] * G : (bs[-1] + 1) * G],
                        vt_sb[:, :, bcols(bs[0], bs[-1] + 1)],
                        vt_sb[:, :, bcols(bs[0], bs[-1] + 1)],
                    )

        # ---- PE node matmuls
        def pair_of(b):
            return (0, b) if b < NP else (1, b - NP)

        def emit_mms(order, ps, lhs_for, t_of):
            first_seen = {0: True, 1: True}
            remaining = {0: sum(1 for b in order if b < NP),
                         1: sum(1 for b in order if b >= NP)}
            for b in order:
                p, r = pair_of(b)
                remaining[p] -= 1
                with W(t_of(b)):
                    for a in range(A):
                        nc.tensor.matmul(
                            ps[p][:],
                            lhs_for(a, r),
                            (vsq[:, a, b * G : (b + 1) * G]
                             if ps is sq_ps
                             else vt_sb[:, a, bcols(b, b + 1)]),
                            start=(first_seen[p] and a == 0),
                            stop=(remaining[p] == 0 and a == 1),
                        )
                first_seen[p] = False

        def emit_split(pre, dots, post):
            ssq_all = pre + post
            fs_d = {0: True, 1: True}
            fs_q = {0: True, 1: True}
            rem_d = {0: sum(1 for b in dots if b < NP),
                     1: sum(1 for b in dots if b >= NP)}
            rem_q = {0: sum(1 for b in ssq_all if b < NP),
                     1: sum(1 for b in ssq_all if b >= NP)}

            def one(b, ps, fs, rem, lhs_for, t):
                p, r = pair_of(b)
                rem[p] -= 1
                with W(t):
                    for a in range(A):
                        nc.tensor.matmul(
                            ps[p][:],
                            lhs_for(a, r),
                            (vsq[:, a, b * G : (b + 1) * G]
                             if ps is sq_ps
                             else vt_sb[:, a, bcols(b, b + 1)]),
                            start=(fs[p] and a == 0),
                            stop=(rem[p] == 0 and a == 1),
                        )
                fs[p] = False

            dl = lambda a, r: dotw[:, a, r * NP : (r + 1) * NP]
            ql = lambda a, r: eye2d[:, r * NP : (r + 1) * NP]
            for b in pre:
                one(b, sq_ps, fs_q, rem_q, ql, ARRIVE[b] + 0.7)
            for b in dots:
                one(b, dot_ps, fs_d, rem_d, dl, max(S_READY, ARRIVE[b] + 0.2))
            for b in post:
                one(b, sq_ps, fs_q, rem_q, ql, max(S_READY, ARRIVE[b] + 0.7))

        emit_split([0, 1, 2], DOT_ORDER, [3, 4, 6, 7, 8, 9, 10, 11, 5, 12, 13])
        # tails
        for p, t_tail in ((0, 23.3), (1, 23.8)):
            with W(t_tail):
                nc.scalar.activation(isv[p][:], sq_ps[p][:], ARSQRT)
                nc.vector.tensor_mul(res[p][:], dot_ps[p][:], isv[p][:])
                nc.sync.dma_start(out_v[p * NP : (p + 1) * NP, :], res[p][:])

    nc.compile()
    return nc


def _get_nc():
    if "nc" not in _cache:
        _cache["nc"] = _build()
    return _cache["nc"]


def _host_inputs(target, node_emb):
    tgt_bf = np.ascontiguousarray(np.asarray(target, dtype=np.float32)).astype(BF)
    node_emb = np.asarray(node_emb, dtype=np.float32)

    in_maps = []
    for c in range(N_CORES):
        shard = np.empty((NPAD, D), dtype=np.float32)
        shard[:NPC] = node_emb[c * NPC : (c + 1) * NPC]
        shard[NPC:] = node_emb[: NPAD - NPC]  # pad with real rows (no 0-norm)
        vtp = np.ascontiguousarray(shard.T.astype(BF))
        in_maps.append({"target": tgt_bf, "vt": vtp})
    return in_maps


def run(pred, target, node_emb, trace=False, **trace_kwargs):
    """Returns (full_output [50000] f32, BassKernelResults)."""
    nc = _get_nc()
    in_maps = _host_inputs(target, node_emb)
    res = bass_utils.run_bass_kernel_spmd(
        nc, in_maps, list(range(N_CORES)), trace=trace, **trace_kwargs
    )
    parts = [res.results[c]["out"][:NPC] for c in range(N_CORES)]
    return np.concatenate(parts).astype(np.float32), res


def kernel(pred, target, node_emb):
    out, _ = run(pred, target, node_emb)
    return out


# revision 24
# speedup vs baseline: 1.0249x; 1.0216x over previous
"""ContrastiveDist kernel for TRN2 (8 NeuronCores, SPMD) -- v4.5.

out[n] = sum_e -(t_e . v_n) / (||t_e|| * ||v_n|| + eps)
       = (s . v_n) / ||v_n||      with s = -sum_e t_e / ||t_e||
(eps shifts the result by ~4e-11 relative -- dropped.)

Schedule design (from the v3 / v4.0-v4.4 traces):
 * THREE DMA queues: SP HWDGE (nc.sync), ACT HWDGE (nc.scalar), GPSIMD
   SWDGE (nc.gpsimd); ~285 GB/s aggregate HBM-bound, round-robin per
   packet.  tgt quarters head both HWDGE rings.  ALL DMA issues carry
   high_priority: they are data-ready at sim t=0, so the static
   scheduler always places them ahead of (mispredicted) compute --
   v4.4's XD issue sat behind phase-A ACT work until 19.6us.
 * eye and dotw are built ON DEVICE with strided memsets/copies
   (diagonal of a [128, 49] tile = stride-8 free-axis slice) -- no eye
   DMA, no eye dependency in the s chain, and the GP ring's first chunk
   is a clean single node block.
 * target entity-major [128e, 16, 256d], 4 quarters: DVE square+reduce
   for q0/q1, GPSIMD square -> DVE reduce for q2/q3.  ACT
   Abs_reciprocal_sqrt emits winv in BF16 directly; the s sign folds
   into ACT Copy(scale=-1) column copies; s_bf copies split per d-half.
 * GP ring: first block immediately, bulk chunks issued after the
   gpsimd phase-A square so tgt keeps the early bandwidth.
 * PE prewarm + filler matmuls bridge the phase-A gaps so the HAM clock
   gate keeps the PE at 2.4 GHz for the node matmuls.
 * psum pairs: blocks 0-6 / 7-13; block-diag lhsT routes block b to its
   psum row; tails are ACT arsqrt [7,448] + one DVE mul; outs on SP.
 * fused DVE tensor_tensor_reduce is NOT used anywhere (locks up HW).
"""

import numpy as np
import ml_dtypes
from contextlib import ExitStack

import concourse.bacc as bacc
import concourse.bass as bass
import concourse.mybir as mybir
import concourse.tile as tile
from concourse import bass_utils

E, D = 2048, 256
N_FULL = 50000
N_CORES = 8
NPC = N_FULL // N_CORES
G = 448
NG = 14
NPAD = G * NG
NP = 7
A = 2
ET = E // 128
EYC = NP * NP            # eye tile columns (on-device)
TQ = 4
H = ET // TQ
WARM_MM = 6

# node chunks: (ring, [blocks], square engine 'V'/'S', est arrival us,
#               issue early?)
CHUNKS = [
    ("G", [0], "S", 10.7, True),            # GE
    ("G", [1, 2], "S", 14.2, True),         # GA
    ("G", [3, 4], "V", 20.0, False),        # GB (issued after gp TT q3)
    ("G", [5], "V", 22.2, False),           # GC
    ("S", [6, 7, 8], "V", 20.0, True),      # SA
    ("A", [9, 10, 11], "S", 20.0, True),    # XA
    ("S", [12], "S", 23.0, True),           # SD
    ("A", [13], "V", 23.0, True),           # XD
]
DOT_ORDER = [0, 1, 2, 3, 4, 6, 7, 8, 9, 10, 11, 5, 12, 13]
SSQ_ORDER = [0, 1, 2, 3, 4, 6, 7, 8, 9, 10, 11, 5, 12, 13]
ARRIVE = {}
for _ring, _bs, _sq, _t, _e in CHUNKS:
    for _b in _bs:
        ARRIVE[_b] = _t
S_READY = 18.4

F32 = mybir.dt.float32
BF16 = mybir.dt.bfloat16
BF = ml_dtypes.bfloat16
ARSQRT = mybir.ActivationFunctionType.Abs_reciprocal_sqrt
SQUARE = mybir.ActivationFunctionType.Square
COPY = mybir.ActivationFunctionType.Copy

_cache = {}


def _build():
    nc = bacc.Bacc(
        "TRN2",
        target_bir_lowering=False,
        debug=False,
        enable_asserts=True,
        num_devices=N_CORES,
    )
    tgt = nc.dram_tensor("target", [E, D], BF16, kind="ExternalInput").ap()
    vt = nc.dram_tensor("vt", [D, NPAD], BF16, kind="ExternalInput").ap()
    out = nc.dram_tensor("out", [NG * G], F32, kind="ExternalOutput").ap()

    with tile.TileContext(nc) as tc, ExitStack() as ctx:
        tpool = ctx.enter_context(tc.tile_pool(name="tgt", bufs=1))
        vpool = ctx.enter_context(tc.tile_pool(name="v", bufs=1))
        spool = ctx.enter_context(tc.tile_pool(name="small", bufs=1))
        scr = ctx.enter_context(tc.tile_pool(name="scr", bufs=1))
        ps_w = ctx.enter_context(tc.tile_pool(name="psw", bufs=1, space="PSUM"))
        ps_sr = ctx.enter_context(tc.tile_pool(name="pssr", bufs=1, space="PSUM"))
        ps_c0 = ctx.enter_context(tc.tile_pool(name="psc0", bufs=1, space="PSUM"))
        ps_c1 = ctx.enter_context(tc.tile_pool(name="psc1", bufs=1, space="PSUM"))
        ps_da = ctx.enter_context(tc.tile_pool(name="psda", bufs=1, space="PSUM"))
        ps_db = ctx.enter_context(tc.tile_pool(name="psdb", bufs=1, space="PSUM"))
        ps_qa = ctx.enter_context(tc.tile_pool(name="psqa", bufs=1, space="PSUM"))
        ps_qb = ctx.enter_context(tc.tile_pool(name="psqb", bufs=1, space="PSUM"))

        tgt_sb = tpool.tile([128, ET, D], BF16, name="tgt_sb")
        tsq = scr.tile([128, ET, D], BF16, name="tsq")
        vt_sb = vpool.tile([128, A, NPAD], BF16, name="vt_sb")
        vsq = vpool.tile([128, A, NPAD], BF16, name="vsq")

        ssq_t = spool.tile([128, ET], F32, name="ssq_t")
        winv = spool.tile([128, ET], BF16, name="winv")
        s_colbf = spool.tile([128, A], BF16, name="s_colbf")
        eye2d = spool.tile([128, EYC], BF16, name="eye2d")
        dotw = spool.tile([128, A, EYC], BF16, name="dotw")
        warm_w = spool.tile([128, 1], BF16, name="warm_w")
        warm_x = spool.tile([128, G], BF16, name="warm_x")
        act_d = spool.tile([1, 1], F32, name="act_d")
        act_s = spool.tile([1, 1], F32, name="act_s")
        isv = [
            spool.tile([NP, G], F32, name="isva"),
            spool.tile([NP, G], F32, name="isvb"),
        ]
        res = [
            spool.tile([NP, G], F32, name="resa"),
            spool.tile([NP, G], F32, name="resb"),
        ]

        warm_ps = ps_w.tile([1, G], F32, name="warm_ps")
        scol_ps = [
            ps_c0.tile([128, 1], F32, name="scol0"),
            ps_c1.tile([128, 1], F32, name="scol1"),
        ]
        dot_ps = [
            ps_da.tile([NP, G], F32, name="dot_psa"),
            ps_db.tile([NP, G], F32, name="dot_psb"),
        ]
        sq_ps = [
            ps_qa.tile([NP, G], F32, name="sq_psa"),
            ps_qb.tile([NP, G], F32, name="sq_psb"),
        ]

        tgt_v = tgt.rearrange("(p j) d -> p j d", j=ET)
        vt_v = vt.rearrange("(a p) n -> p a n", p=128)
        out_v = out.rearrange("(g f) -> g f", f=G)

        def W(us):
            return tc.tile_wait_until(us / 1000.0)

        def bcols(b0, b1):
            return slice(b0 * G, b1 * G)

        ring_eng = {"S": nc.sync, "A": nc.scalar, "G": nc.gpsimd}

        # ---- DMA issues (high_priority: data-ready at t=0, always first)
        with tc.high_priority():
            for q in range(2):
                nc.sync.dma_start(
                    tgt_sb[:, q * H : (q + 1) * H, :],
                    tgt_v[:, q * H : (q + 1) * H, :],
                )
            for q in range(2, 4):
                nc.scalar.dma_start(
                    tgt_sb[:, q * H : (q + 1) * H, :],
                    tgt_v[:, q * H : (q + 1) * H, :],
                )
            for ring, bs, _sq, _t, early in CHUNKS:
                if not early:
                    continue
                sl = bcols(bs[0], bs[-1] + 1)
                ring_eng[ring].dma_start(vt_sb[:, :, sl], vt_v[:, :, sl])
        nc.vector.memset(act_d[:], 1.0)
        nc.scalar.activation(act_s[:], act_d[:], ARSQRT)  # pins the table load

        # ---- consts / on-device eye + dotw skeleton
        nc.vector.memset(warm_w[:], 1.0)
        nc.vector.memset(warm_x[:], 0.0)
        nc.vector.memset(eye2d[:], 0.0)
        nc.vector.memset(eye2d[:, 0 : EYC : NP + 1], 1.0)
        nc.vector.memset(dotw[:], 0.0)

        # ---- PE prewarm + fillers through the phase-A window
        for _ in range(WARM_MM):
            nc.tensor.matmul(warm_ps[:], warm_w[:], warm_x[:], start=True, stop=True)
        for t_fill in (11.0, 13.0, 14.5, 16.0, 17.6):
            with W(t_fill):
                nc.tensor.matmul(
                    warm_ps[:], warm_w[:], warm_x[:], start=True, stop=True
                )

        # ---- phase A ssq: q0/q1 DVE sq+red, q2 GP sq -> DVE red,
        # q3 split: 2 tiles ACT square+accum, 2 tiles GP sq -> DVE red
        for q in (0, 1):
            sl = slice(q * H, (q + 1) * H)
            nc.vector.tensor_mul(tsq[:, sl, :], tgt_sb[:, sl, :], tgt_sb[:, sl, :])
            with tc.high_priority():
                nc.vector.tensor_reduce(
                    ssq_t[:, sl], tsq[:, sl, :],
                    axis=mybir.AxisListType.X, op=mybir.AluOpType.add,
                )
                nc.scalar.activation(winv[:, sl], ssq_t[:, sl], ARSQRT)
        for q in (2, 3):
            sl = slice(q * H, (q + 1) * H)
            nc.gpsimd.tensor_mul(tsq[:, sl, :], tgt_sb[:, sl, :], tgt_sb[:, sl, :])
            with tc.high_priority():
                nc.vector.tensor_reduce(
                    ssq_t[:, sl], tsq[:, sl, :],
                    axis=mybir.AxisListType.X, op=mybir.AluOpType.add,
                )
                nc.scalar.activation(winv[:, sl], ssq_t[:, sl], ARSQRT)
        # GP bulk chunks issue after both gpsimd squares
        for ring, bs, _sq, _t, early in CHUNKS:
            if early or ring != "G":
                continue
            slc = bcols(bs[0], bs[-1] + 1)
            nc.gpsimd.dma_start(vt_sb[:, :, slc], vt_v[:, :, slc])

        # ---- phase A s-column matmuls (v3 scheme: 128-col weight loads
        # pipeline under the 1-col streams; output is s_col psum direct)
        for j in range(ET):
            for a in range(A):
                nc.tensor.matmul(
                    scol_ps[a][:],
                    tgt_sb[:, j, a * 128 : (a + 1) * 128],
                    winv[:, j : j + 1],
                    start=(j == 0),
                    stop=(j == ET - 1),
                )
        for a in range(A):
            with tc.high_priority():
                nc.scalar.activation(
                    s_colbf[:, a : a + 1], scol_ps[a][:], COPY, scale=-1.0
                )
                # dotw diagonal = -s (stride NP+1 hits [r, r])
                nc.vector.tensor_copy(
                    dotw[:, a, 0 : EYC : NP + 1],
                    s_colbf[:, a : a + 1].broadcast_to([128, NP]),
                )

        # ---- node squares
        for ring, bs, sqe, t_arr, _e in CHUNKS:
            if sqe == "S":
                for b in bs:
                    with W(t_arr + 0.2):
                        nc.scalar.activation(
                            vsq[:, :, b * G : (b + 1) * G],
                            vt_sb[:, :, bcols(b, b + 1)],
                            SQUARE,
                        )
            else:
                with W(t_arr + 0.2):
                    nc.vector.tensor_mul(
                        vsq[:, :, bs[0] * G : (bs[-1] + 1) * G],
                        vt_sb[:, :, bcols(bs[0], bs[-1] + 1)],
                        vt_sb[:, :, bcols(bs[0], bs[-1] + 1)],
                    )

        # ---- PE node matmuls
        def pair_of(b):
            return (0, b) if b < NP else (1, b - NP)

        def emit_mms(order, ps, lhs_for, t_of):
            first_seen = {0: True, 1: True}
            remaining = {0: sum(1 for b in order if b < NP),
                         1: sum(1 for b in order if b >= NP)}
            for b in order:
                p, r = pair_of(b)
                remaining[p] -= 1
                with W(t_of(b)):
                    for a in range(A):
                        nc.tensor.matmul(
                            ps[p][:],
                            lhs_for(a, r),
                            (vsq[:, a, b * G : (b + 1) * G]
                             if ps is sq_ps
                             else vt_sb[:, a, bcols(b, b + 1)]),
                            start=(first_seen[p] and a == 0),
                            stop=(remaining[p] == 0 and a == 1),
                        )
                first_seen[p] = False

        def emit_split(pre, dots, post):
            ssq_all = pre + post
            fs_d = {0: True, 1: True}
            fs_q = {0: True, 1: True}
            rem_d = {0: sum(1 for b in dots if b < NP),
                     1: sum(1 for b in dots if b >= NP)}
            rem_q = {0: sum(1 for b in ssq_all if b < NP),
                     1: sum(1 for b in ssq_all if b >= NP)}

            def one(b, ps, fs, rem, lhs_for, t):
                p, r = pair_of(b)
                rem[p] -= 1
                with W(t):
                    for a in range(A):
                        nc.tensor.matmul(
                            ps[p][:],
                            lhs_for(a, r),
                            (vsq[:, a, b * G : (b + 1) * G]
                             if ps is sq_ps
                             else vt_sb[:, a, bcols(b, b + 1)]),
                            start=(fs[p] and a == 0),
                            stop=(rem[p] == 0 and a == 1),
                        )
                fs[p] = False

            dl = lambda a, r: dotw[:, a, r * NP : (r + 1) * NP]
            ql = lambda a, r: eye2d[:, r * NP : (r + 1) * NP]
            for b in pre:
                one(b, sq_ps, fs_q, rem_q, ql, ARRIVE[b] + 0.7)
            for b in dots:
                one(b, dot_ps, fs_d, rem_d, dl, max(S_READY, ARRIVE[b] + 0.2))
            for b in post:
                one(b, sq_ps, fs_q, rem_q, ql, max(S_READY, ARRIVE[b] + 0.7))

        emit_split([0, 1, 2], DOT_ORDER, [3, 4, 9, 10, 11, 6, 7, 8, 5, 13, 12])
        # tails
        for p, t_tail in ((0, 23.4), (1, 23.9)):
            with W(t_tail):
                nc.scalar.activation(isv[p][:], sq_ps[p][:], ARSQRT)
                nc.vector.tensor_mul(res[p][:], dot_ps[p][:], isv[p][:])
                nc.sync.dma_start(out_v[p * NP : (p + 1) * NP, :], res[p][:])

    nc.compile()
    return nc


def _get_nc():
    if "nc" not in _cache:
        _cache["nc"] = _build()
    return _cache["nc"]


def _host_inputs(target, node_emb):
    tgt_bf = np.ascontiguousarray(np.asarray(target, dtype=np.float32)).astype(BF)
    node_emb = np.asarray(node_emb, dtype=np.float32)

    in_maps = []
    for c in range(N_CORES):
        shard = np.empty((NPAD, D), dtype=np.float32)
        shard[:NPC] = node_emb[c * NPC : (c + 1) * NPC]
        shard[NPC:] = node_emb[: NPAD - NPC]  # pad with real rows (no 0-norm)
        vtp = np.ascontiguousarray(shard.T.astype(BF))
        in_maps.append({"target": tgt_bf, "vt": vtp})
    return in_maps


def run(pred, target, node_emb, trace=False, **trace_kwargs):
    """Returns (full_output [50000] f32, BassKernelResults)."""
    nc = _get_nc()
    in_maps = _host_inputs(target, node_emb)
    res = bass_utils.run_bass_kernel_spmd(
        nc, in_maps, list(range(N_CORES)), trace=trace, **trace_kwargs
    )
    parts = [res.results[c]["out"][:NPC] for c in range(N_CORES)]
    return np.concatenate(parts).astype(np.float32), res


def kernel(pred, target, node_emb):
    out, _ = run(pred, target, node_emb)
    return out
